# revision 8
# baseline (speedup 1.0000x reference)
"""Trainium2 Bass kernel for nn_CrossAtt (dual cross-attention + 3x3 conv + BN + ReLU).

Sharding: 8 cores = (sample s in 0..3) x (h-half in 0..1). Each core computes
its 32 output rows plus a 1-row attention halo on each side (34 rows = 2176
query positions, host-zero-padded so the program is SPMD-uniform), then runs
the 3x3 conv locally. No collectives.

Device layout choices:
- scoresT [m, n] comes straight off the PE (lhsT=k zero-padded to K=128,
  rhs=q), so softmax needs no transpose of the 4096x2176 matrix.
- exp on ScalarE (no max subtraction; |scores| <~ 5 so fp32 exp is safe).
- AV: out^T[n, 257] = expT.T @ [vT | ones]; col 256 accumulates the softmax
  denominator S for free.
- normalize by (gamma * mask / S) as a per-partition scalar; mask zeroes the
  fake padded query rows. PE-transposes the small [n,256] result to [256,n].
- residual + gamma*bv bias are folded into the host-prepared x?r inputs.
- conv3x3 = 9 shifted matmuls over a [512, 35*66] zero-padded cat buffer;
  BN+ReLU fused into one activation (scale=inv, bias=beta per partition).
"""
import sys

if "/opt/trn_rl_repo" not in sys.path:
    sys.path.insert(0, "/opt/trn_rl_repo")

import numpy as np

import concourse.bass as bass
import concourse.bacc as bacc
import concourse.mybir as mybir
import concourse.tile as tile
from concourse.bass import ds, ts
from concourse.bass_utils import run_bass_kernel_spmd

F32 = mybir.dt.float32
EPS = 1e-5
P = 128
C = 256          # channels
M = 4096         # key/value positions (64*64)
NQ = 2176        # query positions per core (34 rows * 64), host padded
NROWS = 35       # cat_pad rows (34 data + 1 zero)
WPAD = 66        # cat_pad row width (64 + 2 zero cols)
ATT_BLOCKS = [(0, 512), (512, 512), (1024, 512), (1536, 512), (2048, 128)]
CONV_WINS = [(1, 512), (513, 512), (1025, 512), (1537, 512), (2049, 62)]

_CACHE = {}


def _wins(total, w):
    return [(i, min(w, total - i)) for i in range(0, total, w)]


def _declare_io(nc):
    t = {}
    inp = lambda name, shape: t.__setitem__(
        name, nc.dram_tensor(name, shape, F32, kind="ExternalInput"))
    out = lambda name, shape: t.__setitem__(
        name, nc.dram_tensor(name, shape, F32, kind="ExternalOutput"))
    inp("x1", [C, M]); inp("x2", [C, M])
    inp("x1q", [C, NQ]); inp("x2q", [C, NQ])
    inp("x1r", [C, NQ]); inp("x2r", [C, NQ])
    inp("maskg", [P, 17])
    inp("wq1T", [P, 2, 32]); inp("wq2T", [P, 2, 32])
    inp("wk1T", [P, 2, 64]); inp("wk2T", [P, 2, 64])
    inp("wv1T", [P, 2, C]); inp("wv2T", [P, 2, C])
    inp("bq1", [32, 1]); inp("bq2", [32, 1])
    inp("bk1", [64, 1]); inp("bk2", [64, 1])
    inp("cinv", [P, 2]); inp("cbeta", [P, 2])
    inp("wct", [3, 3, 2 * C, C])
    inp("ident", [P, P])
    out("feat", [C, 32, 64]); out("o1", [C, 32, 64]); out("o2", [C, 32, 64])
    return t


def _emit(nc, tc, t, ctx):
    big = ctx.enter_context(tc.tile_pool(name="big", bufs=3))
    kqp = ctx.enter_context(tc.tile_pool(name="kq", bufs=1))
    sing = ctx.enter_context(tc.tile_pool(name="sing", bufs=1))
    expp = ctx.enter_context(tc.tile_pool(name="expp", bufs=3))
    normp = ctx.enter_context(tc.tile_pool(name="normp", bufs=3))
    scalp = ctx.enter_context(tc.tile_pool(name="scalp", bufs=4))
    resp = ctx.enter_context(tc.tile_pool(name="resp", bufs=4))
    wcp = ctx.enter_context(tc.tile_pool(name="wcp", bufs=4))
    psA = ctx.enter_context(tc.tile_pool(name="psA", bufs=1, space="PSUM"))
    psS = ctx.enter_context(tc.tile_pool(name="psS", bufs=2, space="PSUM"))
    psT = ctx.enter_context(tc.tile_pool(name="psT", bufs=2, space="PSUM"))

    BIG_SHAPE_BYTES = [P, 4 * NROWS * WPAD]  # cat_pad is the largest big tile

    # ---- constants / weights to SBUF ----
    idt = sing.tile([P, P], F32)
    nc.sync.dma_start(out=idt, in_=t["ident"][:])
    wq_sb, wk_sb, wv_sb, bq_sb, bk_sb = {}, {}, {}, {}, {}
    for b in (1, 2):
        wq_sb[b] = sing.tile([P, 2, 32], F32, tag=f"wq{b}", name=f"wq{b}")
        nc.sync.dma_start(out=wq_sb[b], in_=t[f"wq{b}T"][:])
        wk_sb[b] = sing.tile([P, 2, 64], F32, tag=f"wk{b}", name=f"wk{b}")
        nc.sync.dma_start(out=wk_sb[b], in_=t[f"wk{b}T"][:])
        wv_sb[b] = sing.tile([P, 2, C], F32, tag=f"wv{b}", name=f"wv{b}")
        nc.sync.dma_start(out=wv_sb[b], in_=t[f"wv{b}T"][:])
        bq_sb[b] = sing.tile([32, 1], F32, tag=f"bq{b}", name=f"bq{b}")
        nc.sync.dma_start(out=bq_sb[b], in_=t[f"bq{b}"][:])
        bk_sb[b] = sing.tile([64, 1], F32, tag=f"bk{b}", name=f"bk{b}")
        nc.sync.dma_start(out=bk_sb[b], in_=t[f"bk{b}"][:])
    cinv_sb = sing.tile([P, 2], F32, tag="cinv")
    nc.sync.dma_start(out=cinv_sb, in_=t["cinv"][:])
    cbeta_sb = sing.tile([P, 2], F32, tag="cbeta")
    nc.sync.dma_start(out=cbeta_sb, in_=t["cbeta"][:])
    maskg_sb = sing.tile([P, 17], F32, tag="maskg")
    nc.sync.dma_start(out=maskg_sb, in_=t["maskg"][:])

    # ---- load x1, x2 ----
    def load_x(name):
        x_sb = big.tile(BIG_SHAPE_BYTES, F32, tag="big")
        xv = x_sb[:, : 2 * M].rearrange("p (kc n) -> p kc n", kc=2)
        nc.sync.dma_start(out=xv, in_=t[name][:].rearrange("(kc p) n -> p kc n", p=P))
        return xv

    x1_sb = load_x("x1")
    x2_sb = load_x("x2")

    # ---- k projections: k_b = wk_b @ x_b + bk_b, stored [128(c pad0), 4096] ----
    k_sb = {}
    for b, x_sb in ((1, x1_sb), (2, x2_sb)):
        kp = kqp.tile([P, M], F32, tag=f"k{b}")
        nc.gpsimd.memset(kp[64:P, :], 0.0)
        for w0, ww in _wins(M, 512):
            ps = psS.tile([P, 512], F32, tag="sc")
            for kc in range(2):
                nc.tensor.matmul(ps[0:64, :ww], wk_sb[b][:, kc, :],
                                 x_sb[:, kc, ds(w0, ww)],
                                 start=(kc == 0), stop=(kc == 1))
            nc.vector.tensor_scalar_add(kp[0:64, ds(w0, ww)], ps[0:64, :ww], bk_sb[b])
        k_sb[b] = kp

    # ---- vT projections: vT_b[m, c] = x_b.T @ wv_bT (no bias), plus ones col ----
    def make_vt(x_sb, b):
        vt = big.tile(BIG_SHAPE_BYTES, F32, tag="big")
        vtv = vt[:, : 32 * 257].rearrange("p (mi c) -> p mi c", mi=32)
        nc.vector.memset(vtv[:, :, 256:257], 1.0)
        for mi in range(32):
            ps = psT.tile([P, 256], F32, tag="tp")
            for kc in range(2):
                nc.tensor.matmul(ps, x_sb[:, kc, ts(mi, P)], wv_sb[b][:, kc, :],
                                 start=(kc == 0), stop=(kc == 1))
            nc.vector.tensor_copy(out=vtv[:, mi, 0:256], in_=ps)
        return vtv

    vt1 = make_vt(x1_sb, 1)

    # ---- q projection (shared by both branches): qp [128(c pad0), 2176] ----
    qp = kqp.tile([P, NQ], F32, tag="qp")
    nc.gpsimd.memset(qp[64:P, :], 0.0)

    def q_half(name, b, row0):
        xq = big.tile(BIG_SHAPE_BYTES, F32, tag="big")
        xqv = xq[:, : 2 * NQ].rearrange("p (kc n) -> p kc n", kc=2)
        nc.sync.dma_start(out=xqv, in_=t[name][:].rearrange("(kc p) n -> p kc n", p=P))
        for w0, ww in _wins(NQ, 512):
            ps = psS.tile([P, 512], F32, tag="sc")
            for kc in range(2):
                nc.tensor.matmul(ps[0:32, :ww], wq_sb[b][:, kc, :],
                                 xqv[:, kc, ds(w0, ww)],
                                 start=(kc == 0), stop=(kc == 1))
            nc.vector.tensor_scalar_add(qp[row0:row0 + 32, ds(w0, ww)],
                                        ps[0:32, :ww], bq_sb[b])

    q_half("x1q", 1, 0)      # frees x1 slot after vt1+k1 -> xq reuses it
    vt2 = make_vt(x2_sb, 2)
    q_half("x2q", 2, 32)

    # ---- cat_pad buffer [128, 4, 35*66], zeroed ----
    cat = big.tile(BIG_SHAPE_BYTES, F32, tag="big")
    catv = cat[:].rearrange("p (i f) -> p i f", i=4)
    cat_r = cat[:].rearrange("p (i r w) -> p i r w", i=4, w=WPAD)
    nc.gpsimd.memset(cat[:], 0.0)

    # ---- attention branches ----
    for b, (kp, vtv, xr_name) in enumerate(
            [(k_sb[1], vt1, "x1r"), (k_sb[2], vt2, "x2r")]):
        for n0, nw in ATT_BLOCKS:
            nsub = nw // P
            av = psA.tile([P, 2048], F32, tag="av")
            pend = []  # pipeline: delay AV(mi) until after scores(mi+1)
            for mi in range(32):
                sc = psS.tile([P, 512], F32, tag="sc")
                nc.tensor.matmul(sc[:, :nw], kp[:, ts(mi, P)], qp[:, ds(n0, nw)],
                                 start=True, stop=True)
                ex = expp.tile([P, 512], F32, tag="ex")
                nc.scalar.activation(ex[:, :nw], sc[:, :nw],
                                     mybir.ActivationFunctionType.Exp)
                if pend:
                    pmi, pex = pend.pop()
                    for j in range(nsub):
                        nc.tensor.matmul(av[:, ds(j * 512, 257)],
                                         pex[:, ts(j, P)], vtv[:, pmi, :],
                                         start=(pmi == 0), stop=(pmi == 31))
                pend.append((mi, ex))
            pmi, pex = pend.pop()
            for j in range(nsub):
                nc.tensor.matmul(av[:, ds(j * 512, 257)],
                                 pex[:, ts(j, P)], vtv[:, pmi, :],
                                 start=(pmi == 0), stop=(pmi == 31))

            # epilogue per n-chunk of 128
            for j in range(nsub):
                nch = n0 // P + j
                rs = scalp.tile([P, 1], F32, tag="rs")
                nc.vector.reciprocal(rs, av[:, ds(j * 512 + 256, 1)])
                nc.vector.tensor_mul(out=rs, in0=rs,
                                     in1=maskg_sb[:, ds(nch, 1)])
                nt = normp.tile([P, 256], F32, tag="nt")
                nc.vector.tensor_scalar_mul(nt, av[:, ds(j * 512, 256)], rs)
                tp = psT.tile([P, 256], F32, tag="tp")
                for cc in range(2):
                    nc.tensor.transpose(tp[:, ts(cc, P)], nt[:, ts(cc, P)], idt)
                rt = resp.tile([P, 2, P], F32, tag="rt")
                nc.sync.dma_start(
                    out=rt,
                    in_=t[xr_name][:].rearrange("(cc p) n -> p cc n", p=P)
                    [:, :, ts(nch, P)])
                for cc in range(2):
                    nc.vector.tensor_add(
                        out=cat_r[:, 2 * b + cc, ds(2 * nch, 2), ds(1, 64)],
                        in0=tp[:, ts(cc, P)].rearrange("p (r w) -> p r w", w=64),
                        in1=rt[:, cc, :].rearrange("p (r w) -> p r w", w=64))

        # write out this branch's attention output (rows 1..33 = the 32 real rows)
        ov = t[f"o{b + 1}"][:].rearrange("(cc p) h w -> p cc h w", p=P)
        for cc in range(2):
            nc.sync.dma_start(out=ov[:, cc],
                              in_=cat_r[:, 2 * b + cc, ds(1, 32), ds(1, 64)])

    # ---- conv 3x3 + BN + ReLU ----
    feat = big.tile(BIG_SHAPE_BYTES, F32, tag="big")
    featv = feat[:, : 2 * 2112].rearrange("p (o f) -> p o f", o=2)
    feat_r = feat[:, : 2 * 2112].rearrange("p (o r w) -> p o r w", o=2, w=WPAD)
    for oc in range(2):
        avc = psA.tile([P, 2048], F32, tag="av")
        last = psS.tile([P, 512], F32, tag="sc")
        for ic in range(4):
            for tap in range(9):
                wt = wcp.tile([P, P], F32, tag="wt")
                nc.sync.dma_start(
                    out=wt, in_=t["wct"][tap // 3, tap % 3,
                                         ts(ic, P), ts(oc, P)])
                off = (tap // 3) * WPAD + (tap % 3) - 1
                for wi, (ws, ww) in enumerate(CONV_WINS):
                    dst = avc[:, ds(wi * 512, ww)] if wi < 4 else last[:, :ww]
                    nc.tensor.matmul(dst, wt, catv[:, ic, ds(ws + off, ww)],
                                     start=(ic == 0 and tap == 0),
                                     stop=(ic == 3 and tap == 8))
        for wi, (ws, ww) in enumerate(CONV_WINS):
            src = avc[:, ds(wi * 512, ww)] if wi < 4 else last[:, :ww]
            nc.scalar.activation(featv[:, oc, ds(ws, ww)], src,
                                 mybir.ActivationFunctionType.Relu,
                                 bias=cbeta_sb[:, ds(oc, 1)],
                                 scale=cinv_sb[:, ds(oc, 1)])
    fv = t["feat"][:].rearrange("(cc p) h w -> p cc h w", p=P)
    for oc in range(2):
        nc.sync.dma_start(out=fv[:, oc], in_=feat_r[:, oc, :, ds(1, 64)])


def _build():
    if "nc" in _CACHE:
        return _CACHE["nc"]
    nc = bacc.Bacc(None, target_bir_lowering=False)
    t = _declare_io(nc)
    from contextlib import ExitStack
    with tile.TileContext(nc) as tc, ExitStack() as ctx:
        _emit(nc, tc, t, ctx)
    nc.finalize()
    _CACHE["nc"] = nc
    return nc


def _prep_host(inputs):
    d = {k: np.ascontiguousarray(np.asarray(v, np.float32)) for k, v in inputs.items()}
    gamma = float(d["gamma"].reshape(-1)[0])
    inv = d["bn_scale"] / np.sqrt(d["bn_var"] + EPS)
    beta = d["bn_bias"] - d["bn_mean"] * inv

    def chunked(w):  # [256, o] -> [128, 2, o]
        return np.ascontiguousarray(w.reshape(2, P, -1).transpose(1, 0, 2))

    shared = {
        "wq1T": chunked(d["wq1"].T), "wq2T": chunked(d["wq2"].T),
        "wk1T": chunked(d["wk1"].T), "wk2T": chunked(d["wk2"].T),
        "wv1T": chunked(d["wv1"].T), "wv2T": chunked(d["wv2"].T),
        "bq1": d["bq1"].reshape(32, 1).copy(), "bq2": d["bq2"].reshape(32, 1).copy(),
        "bk1": d["bk1"].reshape(64, 1).copy(), "bk2": d["bk2"].reshape(64, 1).copy(),
        "cinv": np.ascontiguousarray(inv.reshape(2, P).T),
        "cbeta": np.ascontiguousarray(beta.reshape(2, P).T),
        "wct": np.ascontiguousarray(d["w_cat"].transpose(2, 3, 1, 0)),
        "ident": np.eye(P, dtype=np.float32),
    }
    gbv = {1: gamma * d["bv1"], 2: gamma * d["bv2"]}

    in_maps = []
    for core in range(8):
        s, half = core // 2, core % 2
        h0 = 32 * half
        x1 = np.ascontiguousarray(d["input1"][s].reshape(C, M))
        x2 = np.ascontiguousarray(d["input2"][s].reshape(C, M))
        n_lo, n_hi = (h0 - 1) * 64, (h0 + 33) * 64
        lo_pad, hi_pad = max(0, -n_lo), max(0, n_hi - M)
        sl = slice(n_lo + lo_pad, n_hi - hi_pad)

        def pad_slice(x, add=None):
            o = np.zeros((C, NQ), np.float32)
            body = x[:, sl]
            if add is not None:
                body = body + add[:, None]
            o[:, lo_pad:NQ - hi_pad] = body
            return o

        maskg = np.zeros(NQ, np.float32)
        maskg[lo_pad:NQ - hi_pad] = gamma
        m = dict(shared)
        m.update({
            "x1": x1, "x2": x2,
            "x1q": pad_slice(x1), "x2q": pad_slice(x2),
            "x1r": pad_slice(x1, gbv[1]), "x2r": pad_slice(x2, gbv[2]),
            "maskg": np.ascontiguousarray(maskg.reshape(17, P).T),
        })
        in_maps.append(m)
    return in_maps


def kernel(**inputs):
    nc = _build()
    in_maps = _prep_host(inputs)
    res = run_bass_kernel_spmd(nc, in_maps, core_ids=list(range(8)))
    _CACHE["last_results"] = res
    feat = np.zeros((4, C, 64, 64), np.float32)
    o1 = np.zeros((4, C, 64, 64), np.float32)
    o2 = np.zeros((4, C, 64, 64), np.float32)
    for core in range(8):
        s, half = core // 2, core % 2
        r = res.results[core]
        feat[s, :, 32 * half:32 * half + 32] = r["feat"]
        o1[s, :, 32 * half:32 * half + 32] = r["o1"]
        o2[s, :, 32 * half:32 * half + 32] = r["o2"]
    return (feat, o1, o2)
